# revision 32
# baseline (speedup 1.0000x reference)
"""Trainium2 Bass kernel for nn_Net_88381837017215 (2-layer GCN message passing).

  h = relu(A @ (features @ W1)); o = softmax(relu(A @ (h @ W2)))

Strategy (8 NeuronCores, SPMD, 3 launches with host gather between):
- Nodes row-sharded: core c owns rows [c*12500,(c+1)*12500), padded to 12544
  (98 windows x 128). Global padded tables: 100352 rows.
- Launch A: x1 = features @ W1 per shard (fp32 PSUM, fp16 out).
  Host concatenates the 8 shards into the full x1 table.
- Launch B: spmm1 + relu + dense2. Edges grouped by owner row-window (128 dst
  nodes) and source-chunk (4 chunks of 25088 table rows so gather indices fit
  int16); each (window,chunk) padded to quota[chunk] tiles of 128 edges. Per
  super-block of SB=7 windows, one bulk dma_gather per chunk fetches the edge
  source rows (fp16, 256B each). Segment-sum via one-hot matmuls:
  S[e,n] = val[e] * (row_local[e]==n) built fp16 with block DVE ops; PE
  accumulates msgs.T @ S into PSUM (output lands transposed, feeding h @ W2
  directly without an explicit transpose). Host concatenates x2 shards.
- Launch C: spmm2 (S.T @ msgs) + relu + on-chip softmax.

kernel(**inputs) takes FULL inputs, shards on host, runs on cores 0-7 via
run_bass_kernel_spmd, returns the FULL [100000, 64] float32 output.
"""
import os
import sys

for _p in ("/opt/trn_rl_repo", "/root/.axon_site/_ro/trn_rl_repo"):
    if os.path.isdir(_p):
        sys.path.insert(0, _p)
        break

import numpy as np

NCORES = 8
N = 100000
P = 128
NSHARD = N // NCORES            # 12500
NWIN = (NSHARD + P - 1) // P    # 98
NPADC = NWIN * P                # 12544
NTOT = NCORES * NPADC           # 100352
NCHUNK = 4
CHROWS = NTOT // NCHUNK         # 25088
SB = 7
NSB = NWIN // SB                # 14
HID, OUT, IN_F = 128, 64, 256
_USE_TRIM = False
_NO_OVERFLOW = True


# ---------------------------------------------------------------- host side

def _build_edge_inputs(edge_row, edge_col, edge_val):
    """Pack edges into per-(window, chunk) regular tiles of QR x 128 plus
    per-(super-block, chunk) shared overflow tiles (window-mixed). Overflow
    pad slots get idx -1 (trailing skip) except for the first 3 super-blocks
    (first use of each gather buffer - must not leave SBUF uninitialized).

    Returns (key, edge_maps) where key = (QR, OVT) selects the compiled
    program and edge_maps[c] = {"idx_all", "s_all"}."""
    core = edge_row // NSHARD
    rlc = edge_row % NSHARD
    win = rlc // P
    row_in_win = rlc % P
    colp = (edge_col // NSHARD) * NPADC + (edge_col % NSHARD)
    chunk = colp // CHROWS
    idx16 = (colp % CHROWS).astype(np.int32)

    key = (core * NWIN + win) * NCHUNK + chunk
    counts = np.bincount(key, minlength=NCORES * NWIN * NCHUNK)
    counts = counts.reshape(NCORES, NWIN, NCHUNK)
    QT = int(np.ceil(counts.max() / P))
    QR = QT if _NO_OVERFLOW else max(1, QT - 1)
    cap = QR * P
    spill_sb = (np.maximum(counts - cap, 0)
                .reshape(NCORES, NSB, SB, NCHUNK).sum(axis=2))
    OVT = int(np.ceil(spill_sb.max() / P))
    if OVT == 0:
        QR, OVT = QT, 0
        cap = QR * P
    TA = NCHUNK * (QR + OVT)            # accumulation tiles per window
    NSLOT = SB * QR + OVT               # gather dst slots per (sb, chunk)

    order = np.argsort(key, kind="stable")
    s_riw = row_in_win[order]
    s_idx = idx16[order]
    s_val = edge_val[order]
    starts = np.zeros(NCORES * NWIN * NCHUNK + 1, np.int64)
    np.cumsum(counts.reshape(-1), out=starts[1:])

    edge_maps = []
    for c in range(NCORES):
        reg_idx = np.zeros((NWIN, NCHUNK, cap), np.int16)
        reg_rl = np.zeros((NWIN, NCHUNK, cap), np.int64)
        reg_val = np.zeros((NWIN, NCHUNK, cap), np.float16)
        ovf_fill = np.zeros((NSB, NCHUNK), np.int64)
        ovf_idx = np.zeros((NSB, NCHUNK, max(OVT, 1) * P), np.int16)
        ovf_rl = np.zeros((NSB, NCHUNK, max(OVT, 1) * P), np.int64)
        ovf_val = np.zeros((NSB, NCHUNK, max(OVT, 1) * P), np.float16)
        ovf_win = np.zeros((NSB, NCHUNK, max(OVT, 1) * P), np.int64)
        for w in range(NWIN):
            sb = w // SB
            g0 = (c * NWIN + w) * NCHUNK
            for k in range(NCHUNK):
                a, b = starts[g0 + k], starts[g0 + k + 1]
                n = b - a
                nr = min(n, cap)
                reg_idx[w, k, :nr] = s_idx[a:a + nr]
                reg_rl[w, k, :nr] = s_riw[a:a + nr]
                reg_val[w, k, :nr] = s_val[a:a + nr]
                if n > nr:
                    f = ovf_fill[sb, k]
                    m = n - nr
                    ovf_idx[sb, k, f:f + m] = s_idx[a + nr:b]
                    ovf_rl[sb, k, f:f + m] = s_riw[a + nr:b]
                    ovf_val[sb, k, f:f + m] = s_val[a + nr:b]
                    ovf_win[sb, k, f:f + m] = w % SB
                    ovf_fill[sb, k] = f + m
        if OVT and _USE_TRIM:
            # trailing-skip pads (safe once every gather buffer was written
            # at least once: gd pool has 3 buffers -> sbs 0..2 must gather
            # their pads for real)
            for sb in range(3, NSB):
                for k in range(NCHUNK):
                    ovf_idx[sb, k, ovf_fill[sb, k]:] = -1

        calls = []
        for sb in range(NSB):
            for k in range(NCHUNK):
                seg = [reg_idx[sb * SB:(sb + 1) * SB, k, :].reshape(-1)]
                if OVT:
                    seg.append(ovf_idx[sb, k])
                calls.append(np.concatenate(seg).reshape(-1, 16).T)
        idx_all = np.tile(np.concatenate(calls, axis=1), (8, 1))

        # Dense one-hot S tiles for the regular part only (j = k*QR + t).
        # Overflow tiles are built on-chip from (rlw, val) compact vectors:
        # S_ovf[e, wl*128+n] = val[e] * (iota == win[e]*128 + rl[e]).
        TR = NCHUNK * QR
        S4 = np.zeros((NWIN, TR, P, P), np.float16)
        rrl = reg_rl.reshape(NWIN, TR, P)
        rvl = reg_val.reshape(NWIN, TR, P)
        np.put_along_axis(S4, rrl[..., None], rvl[..., None], axis=3)
        s_all = np.ascontiguousarray(
            S4.transpose(2, 0, 1, 3).reshape(P, NWIN * TR * P))
        m = {
            "idx_all": np.ascontiguousarray(idx_all, dtype=np.int16),
            "s_all": s_all,
        }
        if OVT:
            rlw = (ovf_win * P + ovf_rl).astype(np.float32)  # [NSB,NCHUNK,OVT*P]
            rlw[ovf_val == 0] = -1.0     # pad slots never match iota
            m["ovw_all"] = np.ascontiguousarray(
                rlw.reshape(NSB, NCHUNK, OVT, P)
                .transpose(3, 0, 1, 2).reshape(P, NSB * NCHUNK * OVT))
            m["ovv_all"] = np.ascontiguousarray(
                ovf_val.astype(np.float32).reshape(NSB, NCHUNK, OVT, P)
                .transpose(3, 0, 1, 2).reshape(P, NSB * NCHUNK * OVT))
        edge_maps.append(m)
    return (QR, OVT), edge_maps


# ------------------------------------------------------------- bass programs

_CACHE = {}


def _bass_mods():
    import concourse.bacc as bacc
    import concourse.tile as tile
    from concourse import mybir
    return bacc, tile, mybir


def _build_prog_a():
    """x1_shard[NPADC, HID] (fp16) = featT.T @ W1 (fp32 accum, fp16 in/out).

    Whole-shard bulk DMAs (2 in, GRP-window out batches) to avoid per-window
    HWDGE issue overhead; PSUM drains alternate scalar/vector engines."""
    bacc, tile, mybir = _bass_mods()
    f32, f16 = mybir.dt.float32, mybir.dt.float16
    AF = mybir.ActivationFunctionType

    nc = bacc.Bacc("TRN2", target_bir_lowering=False, debug=False,
                   num_devices=NCORES)
    featT = nc.dram_tensor("featT", [IN_F, NPADC], f16, kind="ExternalInput")
    W1 = nc.dram_tensor("W1", [IN_F, HID], f16, kind="ExternalInput")
    x1 = nc.dram_tensor("x1", [NPADC, HID], f16, kind="ExternalOutput")
    x1v = x1.rearrange("(w r) c -> w r c", r=P)

    GRP = 14                      # windows per output DMA; 98 = 7 * 14
    with tile.TileContext(nc, num_cores=NCORES) as tc:
        with tc.tile_pool(name="const", bufs=1) as cpool, \
             tc.tile_pool(name="out", bufs=3) as opool, \
             tc.tile_pool(name="ps", bufs=8, space="PSUM") as pspool:
            W1a = cpool.tile([P, HID], f16, tag="W1a")
            nc.sync.dma_start(out=W1a[:], in_=W1[0:P, :])
            W1b = cpool.tile([P, HID], f16, tag="W1b")
            nc.sync.dma_start(out=W1b[:], in_=W1[P:IN_F, :])
            fA = cpool.tile([P, NPADC], f16, tag="fA")
            nc.sync.dma_start(out=fA[:], in_=featT[0:P, :])
            fB = cpool.tile([P, NPADC], f16, tag="fB")
            nc.sync.dma_start(out=fB[:], in_=featT[P:IN_F, :])
            for g in range(NWIN // GRP):
                ot = opool.tile([P, GRP, HID], f16, tag="ot")
                for j in range(GRP):
                    w = g * GRP + j
                    ps = pspool.tile([P, HID], f32, tag="d1")
                    nc.tensor.matmul(ps[:], lhsT=fA[:, w * P:(w + 1) * P],
                                     rhs=W1a[:], start=True, stop=False)
                    nc.tensor.matmul(ps[:], lhsT=fB[:, w * P:(w + 1) * P],
                                     rhs=W1b[:], start=False, stop=True)
                    if j % 2 == 0:
                        nc.scalar.activation(ot[:, j, :], ps[:], AF.Copy)
                    else:
                        nc.vector.tensor_copy(ot[:, j, :], ps[:])
                nc.sync.dma_start(
                    out=x1v[g * GRP:(g + 1) * GRP, :, :].rearrange(
                        "w r c -> r w c"),
                    in_=ot[:])
    nc.compile()
    return nc


def _spmm_phase(nc, tc, mybir, key, table, layer2, W2t, out, iopool, gpool,
                spool, sopool, wpool, pswin, psdense, idx_all, s_all, ovw_all,
                ovv_all, iota):
    """Emit the spmm super-block loop. layer1: hT = relu(msgs.T @ S) then
    x2 = hT.T @ W2 -> out rows (fp16). layer2: o = softmax(relu(S.T @ msgs))
    -> out rows (fp32). Regular S tiles are host-precomputed and streamed;
    overflow S tiles are built on-chip with one tensor_scalar each."""
    f32, f16, i16 = mybir.dt.float32, mybir.dt.float16, mybir.dt.int16
    AF = mybir.ActivationFunctionType
    ALU = mybir.AluOpType

    QR, OVT = key
    TR = NCHUNK * QR
    TA = TR + NCHUNK * OVT
    NSLOT = SB * QR + OVT
    nci = NSLOT * P // 16
    NOV = NCHUNK * OVT
    outv = out.rearrange("(w r) c -> w r c", r=P)

    for sb in range(NSB):
        dsts = []
        for k in range(NCHUNK):
            idxcol = (sb * NCHUNK + k) * nci
            nidx = NSLOT * P
            it = iopool.tile([P, nci], i16, tag=f"idx{k}")
            nc.scalar.dma_start(out=it[:], in_=idx_all[:, idxcol:idxcol + nci])
            dst = gpool.tile([P, NSLOT, P], f16, tag=f"gd{k}")
            nc.gpsimd.dma_gather(
                dst[:], table[k * CHROWS:(k + 1) * CHROWS, :],
                it[:], nidx, nidx, P, single_packet=False,
                queue_num=(sb + k) % 4)
            dsts.append(dst)
        sovs = {}
        if OVT:
            ovwt = iopool.tile([P, NOV], f32, tag="ovw")
            nc.scalar.dma_start(out=ovwt[:],
                                in_=ovw_all[:, sb * NOV:(sb + 1) * NOV])
            ovvt = iopool.tile([P, NOV], f32, tag="ovv")
            nc.scalar.dma_start(out=ovvt[:],
                                in_=ovv_all[:, sb * NOV:(sb + 1) * NOV])
            for k in range(NCHUNK):
                for v in range(OVT):
                    col = k * OVT + v
                    sov = sopool.tile([P, SB * P], f16, tag=f"sov{col}")
                    nc.vector.tensor_scalar(
                        out=sov[:], in0=iota[:],
                        scalar1=ovwt[:, col:col + 1],
                        scalar2=ovvt[:, col:col + 1],
                        op0=ALU.is_equal, op1=ALU.mult)
                    sovs[col] = sov
        ot = wpool.tile([P, SB, OUT], f32 if layer2 else f16, tag="ot")
        for wl in range(SB):
            w = sb * SB + wl
            St = spool.tile([P, TR, P], f16, tag="St")
            nc.sync.dma_start(
                out=St[:], in_=s_all[:, w * TR * P:(w + 1) * TR * P])
            acc = pswin.tile([P, P if not layer2 else OUT], f32, tag="acc")
            mms = [(k * QR + t, dsts[k][:, wl * QR + t, :],
                    St[:, k * QR + t, :])
                   for k in range(NCHUNK) for t in range(QR)]
            mms += [(TR + k * OVT + v, dsts[k][:, SB * QR + v, :],
                     sovs[k * OVT + v][:, wl * P:(wl + 1) * P])
                    for k in range(NCHUNK) for v in range(OVT)]
            for j, dslice, sslice in mms:
                if layer2:
                    nc.tensor.matmul(acc[:], lhsT=sslice,
                                     rhs=dslice[:, 0:OUT],
                                     start=(j == 0), stop=(j == TA - 1))
                else:
                    nc.tensor.matmul(acc[:], lhsT=dslice,
                                     rhs=sslice,
                                     start=(j == 0), stop=(j == TA - 1))
            if not layer2:
                hT = wpool.tile([P, P], f16, tag="hT")
                nc.vector.tensor_scalar_max(hT[:], acc[:], 0.0)
                x2ps = psdense.tile([P, OUT], f32, tag="d2")
                nc.tensor.matmul(x2ps[:], lhsT=hT[:], rhs=W2t[:],
                                 start=True, stop=True)
                nc.vector.tensor_copy(ot[:, wl, :], x2ps[:])
            else:
                r = wpool.tile([P, OUT], f32, tag="r")
                nc.vector.tensor_scalar_max(r[:], acc[:], 0.0)
                nm = wpool.tile([P, 1], f32, tag="nm")
                nc.vector.tensor_reduce(nm[:], r[:],
                                        axis=mybir.AxisListType.X,
                                        op=ALU.max, negate=True)
                ex = wpool.tile([P, OUT], f32, tag="ex")
                se = wpool.tile([P, 1], f32, tag="se")
                nc.scalar.activation(ex[:], r[:], AF.Exp, bias=nm[:],
                                     accum_out=se[:])
                rs = wpool.tile([P, 1], f32, tag="rs")
                nc.vector.reciprocal(rs[:], se[:])
                nc.vector.tensor_scalar_mul(ot[:, wl, :], ex[:], rs[:, 0:1])
        nc.sync.dma_start(
            out=outv[sb * SB:(sb + 1) * SB, :, :].rearrange("w r c -> r w c"),
            in_=ot[:])


def _build_prog_bc(key, layer2):
    bacc, tile, mybir = _bass_mods()
    f32, f16, i16 = mybir.dt.float32, mybir.dt.float16, mybir.dt.int16

    QR, OVT = key
    TR = NCHUNK * QR
    NIDX = NSB * NCHUNK * (SB * QR + OVT) * P // 16

    nc = bacc.Bacc("TRN2", target_bir_lowering=False, debug=False,
                   num_devices=NCORES, num_swdge_queues=4)
    W2 = None
    if layer2:
        table = nc.dram_tensor("x2_full", [NTOT, P], f16,
                               kind="ExternalInput")
        outt = nc.dram_tensor("out", [NPADC, OUT], f32, kind="ExternalOutput")
    else:
        table = nc.dram_tensor("x1_full", [NTOT, HID], f16,
                               kind="ExternalInput")
        outt = nc.dram_tensor("x2", [NPADC, OUT], f16, kind="ExternalOutput")
        W2 = nc.dram_tensor("W2", [HID, OUT], f16, kind="ExternalInput")
    idx_all = nc.dram_tensor("idx_all", [P, NIDX], i16, kind="ExternalInput")
    s_all = nc.dram_tensor("s_all", [P, NWIN * TR * P], f16,
                           kind="ExternalInput")
    ovw_all = ovv_all = None
    if OVT:
        ovw_all = nc.dram_tensor("ovw_all", [P, NSB * NCHUNK * OVT], f32,
                                 kind="ExternalInput")
        ovv_all = nc.dram_tensor("ovv_all", [P, NSB * NCHUNK * OVT], f32,
                                 kind="ExternalInput")

    with tile.TileContext(nc, num_cores=NCORES) as tc:
        with tc.tile_pool(name="const", bufs=1) as cpool, \
             tc.tile_pool(name="io", bufs=4) as iopool, \
             tc.tile_pool(name="gd", bufs=3) as gpool, \
             tc.tile_pool(name="sblk", bufs=8) as spool, \
             tc.tile_pool(name="sov", bufs=2) as sopool, \
             tc.tile_pool(name="wout", bufs=3) as wpool, \
             tc.tile_pool(name="psw", bufs=4, space="PSUM") as pswin, \
             tc.tile_pool(name="psd", bufs=2, space="PSUM") as psdense:
            W2t = None
            if not layer2:
                W2t = cpool.tile([P, OUT], f16, tag="W2t")
                nc.sync.dma_start(out=W2t[:], in_=W2[:])
            iota = None
            if OVT:
                iota = cpool.tile([P, SB * P], f16, tag="iota")
                nc.gpsimd.iota(iota[:], pattern=[[1, SB * P]], base=0,
                               channel_multiplier=0,
                               allow_small_or_imprecise_dtypes=True)
            _spmm_phase(nc, tc, mybir, key, table, layer2, W2t, outt,
                        iopool, gpool, spool, sopool, wpool, pswin, psdense,
                        idx_all, s_all, ovw_all, ovv_all, iota)
    nc.compile()
    return nc


# ------------------------------------------------------------------- kernel

PROFILE = False          # set True (with NTFF hook installed) to trace launches
LAST_PROFILE = []        # [(exec_time_ns, tmpdir), ...] per launch when PROFILE


def _run(prog, maps, cores):
    from concourse.bass_utils import run_bass_kernel_spmd
    kw = {}
    if PROFILE:
        import tempfile
        kw = dict(trace=True, tmpdir=tempfile.mkdtemp(prefix="gnnprof_"))
    r = run_bass_kernel_spmd(prog, maps, cores, **kw)
    if PROFILE:
        LAST_PROFILE.append((r.exec_time_ns, kw.get("tmpdir")))
    return r


def _get_progs(key):
    if key not in _CACHE:
        _CACHE[key] = (_build_prog_a(), _build_prog_bc(key, False),
                       _build_prog_bc(key, True))
    return _CACHE[key]


def kernel(features, edge_row, edge_col, edge_val, W1, W2):
    features = np.asarray(features, dtype=np.float32)
    key, edge_maps = _build_edge_inputs(
        np.asarray(edge_row, dtype=np.int64),
        np.asarray(edge_col, dtype=np.int64),
        np.asarray(edge_val, dtype=np.float32))
    prog_a, prog_b, prog_c = _get_progs(key)
    cores = list(range(NCORES))
    W1f = np.ascontiguousarray(W1, dtype=np.float16)
    W2f = np.ascontiguousarray(W2, dtype=np.float16)

    # launch A: dense1
    a_maps = []
    for c in range(NCORES):
        f = np.zeros((NPADC, IN_F), np.float16)
        f[:NSHARD] = features[c * NSHARD:(c + 1) * NSHARD].astype(np.float16)
        a_maps.append({"featT": np.ascontiguousarray(f.T), "W1": W1f})
    res_a = _run(prog_a, a_maps, cores)
    x1_full = np.concatenate([res_a.results[c]["x1"] for c in range(NCORES)],
                             axis=0)

    # launch B: spmm1 + dense2
    b_maps = [{"x1_full": x1_full, "W2": W2f, **edge_maps[c]}
              for c in range(NCORES)]
    res_b = _run(prog_b, b_maps, cores)
    x2_full = np.zeros((NTOT, P), np.float16)
    x2_full[:, :OUT] = np.concatenate(
        [res_b.results[c]["x2"] for c in range(NCORES)], axis=0)

    # launch C: spmm2 + softmax
    c_maps = [{"x2_full": x2_full, **edge_maps[c]} for c in range(NCORES)]
    res_c = _run(prog_c, c_maps, cores)
    return np.concatenate(
        [res_c.results[c]["out"][:NSHARD] for c in range(NCORES)],
        axis=0).astype(np.float32)



# revision 34
# speedup vs baseline: 1.1895x; 1.1895x over previous
"""Trainium2 Bass kernel for nn_Net_88381837017215 (2-layer GCN message passing).

  h = relu(A @ (features @ W1)); o = softmax(relu(A @ (h @ W2)))

Strategy (8 NeuronCores, SPMD, 3 launches with host gather between):
- Nodes row-sharded: core c owns rows [c*12500,(c+1)*12500), padded to 12544
  (98 windows x 128). Global padded tables: 100352 rows.
- Launch A: x1 = features @ W1 per shard (fp16 in, fp32 PSUM, fp16 out),
  whole-shard bulk DMAs. Host concatenates the 8 shards into the x1 table.
- Launch B: spmm1 + relu + dense2. Edges grouped by owner row-window (128 dst
  nodes) and source-chunk (4 chunks of 25088 table rows so gather indices fit
  int16); each (window,chunk) padded to QR tiles of 128 edges. Per super-block
  of SB=7 windows, one bulk dma_gather per chunk fetches the edge source rows
  (fp16, 256B each). The four per-chunk gathers use SWDGE queues 0-3 so
  descriptor generation runs on all four Q7 core pairs concurrently (this is
  the kernel's critical path: ~8ns/edge/queue-pair of Q7 descriptor-gen).
  idx loads ride the Scalar engine's HWDGE so they never queue behind the
  S-tile streams on Sync. Segment-sum via one-hot matmuls: the one-hot
  S[e,n] = val[e]*(row_local[e]==n) tiles are HOST-precomputed fp16 and
  streamed from HBM (keeps the Vector engine idle - DVE 2-port work locks
  GpSimd out of its SBUF port and throttles descriptor generation). PE
  accumulates msgs.T @ S into PSUM (hT lands transposed, feeding h @ W2
  directly). Host concatenates x2 shards into a 256B/row table.
- Launch C: spmm2 (S.T @ msgs) + relu + on-chip softmax.

kernel(**inputs) takes FULL inputs, shards on host, runs on cores 0-7 via
run_bass_kernel_spmd, returns the FULL [100000, 64] float32 output.
"""
import os
import sys

for _p in ("/opt/trn_rl_repo", "/root/.axon_site/_ro/trn_rl_repo"):
    if os.path.isdir(_p):
        sys.path.insert(0, _p)
        break

import numpy as np

NCORES = 8
N = 100000
P = 128
NSHARD = N // NCORES            # 12500
NWIN = (NSHARD + P - 1) // P    # 98
NPADC = NWIN * P                # 12544
NTOT = NCORES * NPADC           # 100352
NCHUNK = 4
CHROWS = NTOT // NCHUNK         # 25088
SB = 7
NSB = NWIN // SB                # 14
HID, OUT, IN_F = 128, 64, 256
_USE_TRIM = False
_NO_OVERFLOW = True


# ---------------------------------------------------------------- host side

def _build_edge_inputs(edge_row, edge_col, edge_val):
    """Pack edges into per-(window, chunk) regular tiles of QR x 128 plus
    per-(super-block, chunk) shared overflow tiles (window-mixed). Overflow
    pad slots get idx -1 (trailing skip) except for the first 3 super-blocks
    (first use of each gather buffer - must not leave SBUF uninitialized).

    Returns (key, edge_maps) where key = (QR, OVT) selects the compiled
    program and edge_maps[c] = {"idx_all", "s_all"}."""
    core = edge_row // NSHARD
    rlc = edge_row % NSHARD
    win = rlc // P
    row_in_win = rlc % P
    colp = (edge_col // NSHARD) * NPADC + (edge_col % NSHARD)
    chunk = colp // CHROWS
    idx16 = (colp % CHROWS).astype(np.int32)

    key = (core * NWIN + win) * NCHUNK + chunk
    counts = np.bincount(key, minlength=NCORES * NWIN * NCHUNK)
    counts = counts.reshape(NCORES, NWIN, NCHUNK)
    QT = int(np.ceil(counts.max() / P))
    QR = QT if _NO_OVERFLOW else max(1, QT - 1)
    cap = QR * P
    spill_sb = (np.maximum(counts - cap, 0)
                .reshape(NCORES, NSB, SB, NCHUNK).sum(axis=2))
    OVT = int(np.ceil(spill_sb.max() / P))
    if OVT == 0:
        QR, OVT = QT, 0
        cap = QR * P
    TA = NCHUNK * (QR + OVT)            # accumulation tiles per window
    NSLOT = SB * QR + OVT               # gather dst slots per (sb, chunk)

    order = np.argsort(key, kind="stable")
    s_riw = row_in_win[order]
    s_idx = idx16[order]
    s_val = edge_val[order]
    starts = np.zeros(NCORES * NWIN * NCHUNK + 1, np.int64)
    np.cumsum(counts.reshape(-1), out=starts[1:])

    edge_maps = []
    for c in range(NCORES):
        reg_idx = np.zeros((NWIN, NCHUNK, cap), np.int16)
        reg_rl = np.zeros((NWIN, NCHUNK, cap), np.int64)
        reg_val = np.zeros((NWIN, NCHUNK, cap), np.float16)
        ovf_fill = np.zeros((NSB, NCHUNK), np.int64)
        ovf_idx = np.zeros((NSB, NCHUNK, max(OVT, 1) * P), np.int16)
        ovf_rl = np.zeros((NSB, NCHUNK, max(OVT, 1) * P), np.int64)
        ovf_val = np.zeros((NSB, NCHUNK, max(OVT, 1) * P), np.float16)
        ovf_win = np.zeros((NSB, NCHUNK, max(OVT, 1) * P), np.int64)
        for w in range(NWIN):
            sb = w // SB
            g0 = (c * NWIN + w) * NCHUNK
            for k in range(NCHUNK):
                a, b = starts[g0 + k], starts[g0 + k + 1]
                n = b - a
                nr = min(n, cap)
                reg_idx[w, k, :nr] = s_idx[a:a + nr]
                reg_rl[w, k, :nr] = s_riw[a:a + nr]
                reg_val[w, k, :nr] = s_val[a:a + nr]
                if n > nr:
                    f = ovf_fill[sb, k]
                    m = n - nr
                    ovf_idx[sb, k, f:f + m] = s_idx[a + nr:b]
                    ovf_rl[sb, k, f:f + m] = s_riw[a + nr:b]
                    ovf_val[sb, k, f:f + m] = s_val[a + nr:b]
                    ovf_win[sb, k, f:f + m] = w % SB
                    ovf_fill[sb, k] = f + m
        if OVT and _USE_TRIM:
            # trailing-skip pads (safe once every gather buffer was written
            # at least once: gd pool has 3 buffers -> sbs 0..2 must gather
            # their pads for real)
            for sb in range(3, NSB):
                for k in range(NCHUNK):
                    ovf_idx[sb, k, ovf_fill[sb, k]:] = -1

        calls = []
        for sb in range(NSB):
            for k in range(NCHUNK):
                seg = [reg_idx[sb * SB:(sb + 1) * SB, k, :].reshape(-1)]
                if OVT:
                    seg.append(ovf_idx[sb, k])
                calls.append(np.concatenate(seg).reshape(-1, 16).T)
        idx_all = np.tile(np.concatenate(calls, axis=1), (8, 1))

        # Dense one-hot S tiles for the regular part only (j = k*QR + t).
        # Overflow tiles are built on-chip from (rlw, val) compact vectors:
        # S_ovf[e, wl*128+n] = val[e] * (iota == win[e]*128 + rl[e]).
        TR = NCHUNK * QR
        S4 = np.zeros((NWIN, TR, P, P), np.float16)
        rrl = reg_rl.reshape(NWIN, TR, P)
        rvl = reg_val.reshape(NWIN, TR, P)
        np.put_along_axis(S4, rrl[..., None], rvl[..., None], axis=3)
        s_all = np.ascontiguousarray(
            S4.transpose(2, 0, 1, 3).reshape(P, NWIN * TR * P))
        m = {
            "idx_all": np.ascontiguousarray(idx_all, dtype=np.int16),
            "s_all": s_all,
        }
        if OVT:
            rlw = (ovf_win * P + ovf_rl).astype(np.float32)  # [NSB,NCHUNK,OVT*P]
            rlw[ovf_val == 0] = -1.0     # pad slots never match iota
            m["ovw_all"] = np.ascontiguousarray(
                rlw.reshape(NSB, NCHUNK, OVT, P)
                .transpose(3, 0, 1, 2).reshape(P, NSB * NCHUNK * OVT))
            m["ovv_all"] = np.ascontiguousarray(
                ovf_val.astype(np.float32).reshape(NSB, NCHUNK, OVT, P)
                .transpose(3, 0, 1, 2).reshape(P, NSB * NCHUNK * OVT))
        edge_maps.append(m)
    return (QR, OVT), edge_maps


# ------------------------------------------------------------- bass programs

_CACHE = {}


def _bass_mods():
    import concourse.bacc as bacc
    import concourse.tile as tile
    from concourse import mybir
    return bacc, tile, mybir


def _build_prog_a():
    """x1_shard[NPADC, HID] (fp16) = featT.T @ W1 (fp32 accum, fp16 in/out).

    Whole-shard bulk DMAs (2 in, GRP-window out batches) to avoid per-window
    HWDGE issue overhead; PSUM drains alternate scalar/vector engines."""
    bacc, tile, mybir = _bass_mods()
    f32, f16 = mybir.dt.float32, mybir.dt.float16
    AF = mybir.ActivationFunctionType

    nc = bacc.Bacc("TRN2", target_bir_lowering=False, debug=False,
                   num_devices=NCORES)
    featT = nc.dram_tensor("featT", [IN_F, NPADC], f16, kind="ExternalInput")
    W1 = nc.dram_tensor("W1", [IN_F, HID], f16, kind="ExternalInput")
    x1 = nc.dram_tensor("x1", [NPADC, HID], f16, kind="ExternalOutput")
    x1v = x1.rearrange("(w r) c -> w r c", r=P)

    GRP = 14                      # windows per output DMA; 98 = 7 * 14
    with tile.TileContext(nc, num_cores=NCORES) as tc:
        with tc.tile_pool(name="const", bufs=1) as cpool, \
             tc.tile_pool(name="out", bufs=3) as opool, \
             tc.tile_pool(name="ps", bufs=8, space="PSUM") as pspool:
            W1a = cpool.tile([P, HID], f16, tag="W1a")
            nc.sync.dma_start(out=W1a[:], in_=W1[0:P, :])
            W1b = cpool.tile([P, HID], f16, tag="W1b")
            nc.sync.dma_start(out=W1b[:], in_=W1[P:IN_F, :])
            fA = cpool.tile([P, NPADC], f16, tag="fA")
            nc.sync.dma_start(out=fA[:], in_=featT[0:P, :])
            fB = cpool.tile([P, NPADC], f16, tag="fB")
            nc.sync.dma_start(out=fB[:], in_=featT[P:IN_F, :])
            for g in range(NWIN // GRP):
                ot = opool.tile([P, GRP, HID], f16, tag="ot")
                for j in range(GRP):
                    w = g * GRP + j
                    ps = pspool.tile([P, HID], f32, tag="d1")
                    nc.tensor.matmul(ps[:], lhsT=fA[:, w * P:(w + 1) * P],
                                     rhs=W1a[:], start=True, stop=False)
                    nc.tensor.matmul(ps[:], lhsT=fB[:, w * P:(w + 1) * P],
                                     rhs=W1b[:], start=False, stop=True)
                    if j % 2 == 0:
                        nc.scalar.activation(ot[:, j, :], ps[:], AF.Copy)
                    else:
                        nc.vector.tensor_copy(ot[:, j, :], ps[:])
                nc.sync.dma_start(
                    out=x1v[g * GRP:(g + 1) * GRP, :, :].rearrange(
                        "w r c -> r w c"),
                    in_=ot[:])
    nc.compile()
    return nc


def _spmm_phase(nc, tc, mybir, key, table, layer2, W2t, out, iopool, gpool,
                spool, sopool, wpool, pswin, psdense, idx_all, s_all, ovw_all,
                ovv_all, iota):
    """Emit the spmm super-block loop. layer1: hT = relu(msgs.T @ S) then
    x2 = hT.T @ W2 -> out rows (fp16). layer2: o = softmax(relu(S.T @ msgs))
    -> out rows (fp32). Regular S tiles are host-precomputed and streamed;
    overflow S tiles are built on-chip with one tensor_scalar each."""
    f32, f16, i16 = mybir.dt.float32, mybir.dt.float16, mybir.dt.int16
    AF = mybir.ActivationFunctionType
    ALU = mybir.AluOpType

    QR, OVT = key
    TR = NCHUNK * QR
    TA = TR + NCHUNK * OVT
    NSLOT = SB * QR + OVT
    nci = NSLOT * P // 16
    NOV = NCHUNK * OVT
    outv = out.rearrange("(w r) c -> w r c", r=P)

    for sb in range(NSB):
        dsts = []
        for k in range(NCHUNK):
            idxcol = (sb * NCHUNK + k) * nci
            nidx = NSLOT * P
            it = iopool.tile([P, nci], i16, tag=f"idx{k}")
            nc.scalar.dma_start(out=it[:], in_=idx_all[:, idxcol:idxcol + nci])
            dst = gpool.tile([P, NSLOT, P], f16, tag=f"gd{k}")
            nc.gpsimd.dma_gather(
                dst[:], table[k * CHROWS:(k + 1) * CHROWS, :],
                it[:], nidx, nidx, P, single_packet=False,
                queue_num=(sb + k) % 4)
            dsts.append(dst)
        sovs = {}
        if OVT:
            ovwt = iopool.tile([P, NOV], f32, tag="ovw")
            nc.scalar.dma_start(out=ovwt[:],
                                in_=ovw_all[:, sb * NOV:(sb + 1) * NOV])
            ovvt = iopool.tile([P, NOV], f32, tag="ovv")
            nc.scalar.dma_start(out=ovvt[:],
                                in_=ovv_all[:, sb * NOV:(sb + 1) * NOV])
            for k in range(NCHUNK):
                for v in range(OVT):
                    col = k * OVT + v
                    sov = sopool.tile([P, SB * P], f16, tag=f"sov{col}")
                    nc.vector.tensor_scalar(
                        out=sov[:], in0=iota[:],
                        scalar1=ovwt[:, col:col + 1],
                        scalar2=ovvt[:, col:col + 1],
                        op0=ALU.is_equal, op1=ALU.mult)
                    sovs[col] = sov
        ot = wpool.tile([P, SB, OUT], f32 if layer2 else f16, tag="ot")
        for wl in range(SB):
            w = sb * SB + wl
            St = spool.tile([P, TR, P], f16, tag="St")
            nc.sync.dma_start(
                out=St[:], in_=s_all[:, w * TR * P:(w + 1) * TR * P])
            acc = pswin.tile([P, P if not layer2 else OUT], f32, tag="acc")
            mms = [(k * QR + t, dsts[k][:, wl * QR + t, :],
                    St[:, k * QR + t, :])
                   for k in range(NCHUNK) for t in range(QR)]
            mms += [(TR + k * OVT + v, dsts[k][:, SB * QR + v, :],
                     sovs[k * OVT + v][:, wl * P:(wl + 1) * P])
                    for k in range(NCHUNK) for v in range(OVT)]
            for j, dslice, sslice in mms:
                if layer2:
                    nc.tensor.matmul(acc[:], lhsT=sslice,
                                     rhs=dslice[:, 0:OUT],
                                     start=(j == 0), stop=(j == TA - 1))
                else:
                    nc.tensor.matmul(acc[:], lhsT=dslice,
                                     rhs=sslice,
                                     start=(j == 0), stop=(j == TA - 1))
            if not layer2:
                hT = wpool.tile([P, P], f16, tag="hT")
                nc.scalar.activation(hT[:], acc[:], AF.Relu)
                x2ps = psdense.tile([P, OUT], f32, tag="d2")
                nc.tensor.matmul(x2ps[:], lhsT=hT[:], rhs=W2t[:],
                                 start=True, stop=True)
                nc.scalar.activation(ot[:, wl, :], x2ps[:], AF.Copy)
            else:
                r = wpool.tile([P, OUT], f32, tag="r")
                nc.scalar.activation(r[:], acc[:], AF.Relu)
                nm = wpool.tile([P, 1], f32, tag="nm")
                nc.vector.tensor_reduce(nm[:], r[:],
                                        axis=mybir.AxisListType.X,
                                        op=ALU.max, negate=True)
                ex = wpool.tile([P, OUT], f32, tag="ex")
                se = wpool.tile([P, 1], f32, tag="se")
                nc.scalar.activation(ex[:], r[:], AF.Exp, bias=nm[:],
                                     accum_out=se[:])
                rs = wpool.tile([P, 1], f32, tag="rs")
                nc.vector.reciprocal(rs[:], se[:])
                nc.scalar.activation(ot[:, wl, :], ex[:], AF.Copy, scale=rs[:])
        nc.sync.dma_start(
            out=outv[sb * SB:(sb + 1) * SB, :, :].rearrange("w r c -> r w c"),
            in_=ot[:])


def _build_prog_bc(key, layer2):
    bacc, tile, mybir = _bass_mods()
    f32, f16, i16 = mybir.dt.float32, mybir.dt.float16, mybir.dt.int16

    QR, OVT = key
    TR = NCHUNK * QR
    NIDX = NSB * NCHUNK * (SB * QR + OVT) * P // 16

    nc = bacc.Bacc("TRN2", target_bir_lowering=False, debug=False,
                   num_devices=NCORES, num_swdge_queues=4)
    W2 = None
    if layer2:
        table = nc.dram_tensor("x2_full", [NTOT, P], f16,
                               kind="ExternalInput")
        outt = nc.dram_tensor("out", [NPADC, OUT], f32, kind="ExternalOutput")
    else:
        table = nc.dram_tensor("x1_full", [NTOT, HID], f16,
                               kind="ExternalInput")
        outt = nc.dram_tensor("x2", [NPADC, OUT], f16, kind="ExternalOutput")
        W2 = nc.dram_tensor("W2", [HID, OUT], f16, kind="ExternalInput")
    idx_all = nc.dram_tensor("idx_all", [P, NIDX], i16, kind="ExternalInput")
    s_all = nc.dram_tensor("s_all", [P, NWIN * TR * P], f16,
                           kind="ExternalInput")
    ovw_all = ovv_all = None
    if OVT:
        ovw_all = nc.dram_tensor("ovw_all", [P, NSB * NCHUNK * OVT], f32,
                                 kind="ExternalInput")
        ovv_all = nc.dram_tensor("ovv_all", [P, NSB * NCHUNK * OVT], f32,
                                 kind="ExternalInput")

    with tile.TileContext(nc, num_cores=NCORES) as tc:
        with tc.tile_pool(name="const", bufs=1) as cpool, \
             tc.tile_pool(name="io", bufs=4) as iopool, \
             tc.tile_pool(name="gd", bufs=3) as gpool, \
             tc.tile_pool(name="sblk", bufs=8) as spool, \
             tc.tile_pool(name="sov", bufs=2) as sopool, \
             tc.tile_pool(name="wout", bufs=3) as wpool, \
             tc.tile_pool(name="psw", bufs=4, space="PSUM") as pswin, \
             tc.tile_pool(name="psd", bufs=2, space="PSUM") as psdense:
            W2t = None
            if not layer2:
                W2t = cpool.tile([P, OUT], f16, tag="W2t")
                nc.sync.dma_start(out=W2t[:], in_=W2[:])
            iota = None
            if OVT:
                iota = cpool.tile([P, SB * P], f16, tag="iota")
                nc.gpsimd.iota(iota[:], pattern=[[1, SB * P]], base=0,
                               channel_multiplier=0,
                               allow_small_or_imprecise_dtypes=True)
            _spmm_phase(nc, tc, mybir, key, table, layer2, W2t, outt,
                        iopool, gpool, spool, sopool, wpool, pswin, psdense,
                        idx_all, s_all, ovw_all, ovv_all, iota)
    nc.compile()
    return nc


# ------------------------------------------------------------------- kernel

PROFILE = False          # set True (with NTFF hook installed) to trace launches
LAST_PROFILE = []        # [(exec_time_ns, tmpdir), ...] per launch when PROFILE


def _run(prog, maps, cores):
    from concourse.bass_utils import run_bass_kernel_spmd
    kw = {}
    if PROFILE:
        import tempfile
        kw = dict(trace=True, tmpdir=tempfile.mkdtemp(prefix="gnnprof_"))
    r = run_bass_kernel_spmd(prog, maps, cores, **kw)
    if PROFILE:
        LAST_PROFILE.append((r.exec_time_ns, kw.get("tmpdir")))
    return r


def _get_progs(key):
    if key not in _CACHE:
        _CACHE[key] = (_build_prog_a(), _build_prog_bc(key, False),
                       _build_prog_bc(key, True))
    return _CACHE[key]


def kernel(features, edge_row, edge_col, edge_val, W1, W2):
    features = np.asarray(features, dtype=np.float32)
    key, edge_maps = _build_edge_inputs(
        np.asarray(edge_row, dtype=np.int64),
        np.asarray(edge_col, dtype=np.int64),
        np.asarray(edge_val, dtype=np.float32))
    prog_a, prog_b, prog_c = _get_progs(key)
    cores = list(range(NCORES))
    W1f = np.ascontiguousarray(W1, dtype=np.float16)
    W2f = np.ascontiguousarray(W2, dtype=np.float16)

    # launch A: dense1
    a_maps = []
    for c in range(NCORES):
        f = np.zeros((NPADC, IN_F), np.float16)
        f[:NSHARD] = features[c * NSHARD:(c + 1) * NSHARD].astype(np.float16)
        a_maps.append({"featT": np.ascontiguousarray(f.T), "W1": W1f})
    res_a = _run(prog_a, a_maps, cores)
    x1_full = np.concatenate([res_a.results[c]["x1"] for c in range(NCORES)],
                             axis=0)

    # launch B: spmm1 + dense2
    b_maps = [{"x1_full": x1_full, "W2": W2f, **edge_maps[c]}
              for c in range(NCORES)]
    res_b = _run(prog_b, b_maps, cores)
    x2_full = np.zeros((NTOT, P), np.float16)
    x2_full[:, :OUT] = np.concatenate(
        [res_b.results[c]["x2"] for c in range(NCORES)], axis=0)

    # launch C: spmm2 + softmax
    c_maps = [{"x2_full": x2_full, **edge_maps[c]} for c in range(NCORES)]
    res_c = _run(prog_c, c_maps, cores)
    return np.concatenate(
        [res_c.results[c]["out"][:NSHARD] for c in range(NCORES)],
        axis=0).astype(np.float32)

